# revision 1
# baseline (speedup 1.0000x reference)
"""Bipartite 2-layer SAGEConv GNN on 8 Trainium2 NeuronCores.

Strategy:
  - Edges sharded by destination range (core c owns dst rows [S*c, S*(c+1))
    for BOTH directions, so layer-2 lin_r terms stay core-local).
  - Per core+direction, dsts are sorted by degree; edges packed into 8-slot
    segments, 16 dst-rows per PSUM block, variable tiles per block
    (schedule = max over cores, so one SPMD program serves all cores).
  - Message gather: dma_gather with a CENTERED table base (idx int16 signed,
    idx = node - N/2) so all 50001 rows are addressable.
  - Segment-sum: PE matmul with constant one-hot lhsT R8 [128 slots, 16 rows]
    accumulated in PSUM per block (no scatter-add races).
  - Layer 2 transform-first: z = x1 @ w2l.T (64 wide) gathered instead of x1.
  - One AllGather per z table; everything else core-local.
  - Degree-permutation undone at DRAM stores via unique-index dma_scatter_add.
"""
import sys
import numpy as np

sys.path.insert(0, "/opt/trn_rl_repo")

# ---------------- problem dims (hardcoded for the harness) ----------------
N = 50000
E = 800000
F_IN = 128
HID = 256
CLS = 64
NCORES = 8

SEG = 4            # slots per segment (one dst's edges per tile-row)
BPD = 32           # dsts per psum block (32-partition alignment for engine ops)
CHUNK_TILES = 8    # tiles per gather call (1024 idx = HW SWDGE ring limit)
SCAT_CHUNK = 512   # rows per scatter-add call (2 read descs per row)


class CFG:
    def __init__(self, n=N, e=E, center=None):
        self.N = n
        self.E = e
        self.S = n // NCORES          # dst rows per core
        self.CENTER = n // 2 if center is None else center  # gather table base row
        self.ZROW = n                 # zero row index (centered: n - CENTER >= 0)
        self.NB = -(-self.S // BPD)   # blocks per direction
        self.RT = -(-self.S // 128)   # 128-row tiles of the slice
        self.SP = self.RT * 128       # padded rows


# ---------------- host-side edge scheduling ----------------

def _prep_dir(src_g, dst_g, c, cfg):
    """Per-core, per-direction metadata."""
    lo = c * cfg.S
    m = (dst_g >= lo) & (dst_g < lo + cfg.S)
    ls = src_g[m].astype(np.int64)
    ld = (dst_g[m] - lo).astype(np.int64)
    deg = np.bincount(ld, minlength=cfg.S)
    pi = np.argsort(-deg, kind="stable").astype(np.int64)
    order = np.argsort(ld, kind="stable")
    ls_s = ls[order]
    starts = np.zeros(cfg.S + 1, np.int64)
    starts[1:] = np.cumsum(deg)
    degp = np.zeros(cfg.NB * BPD, np.int64)
    degp[: cfg.S] = deg[pi]
    treq = np.maximum(
        1, -(-degp.reshape(cfg.NB, BPD).max(1) // SEG)
    ).astype(np.int64)
    return dict(pi=pi, deg=deg, starts=starts, ls_s=ls_s, degp=degp, treq=treq)


def _n_tiles(T):
    return int(T.sum())


def _build_slots(meta, T, cfg):
    """Slot array (src node ids, ZROW for dummies) per the shared schedule.

    Also guarantees every CHUNK_TILES-aligned tile boundary ends with a slot
    whose centered index is non-negative (the SWDGE ucode trims trailing
    negatives), swapping within a dst row -- or dst rows within the block --
    when needed. meta["pi"] is mutated accordingly.
    """
    pi, deg, starts, ls_s = meta["pi"], meta["deg"], meta["starts"], meta["ls_s"]
    total_tiles = int(T.sum())
    out = np.full((total_tiles, BPD, SEG), cfg.ZROW, np.int64)
    row_of_tile = np.zeros(total_tiles, np.int64)   # block index per tile
    t0 = 0
    blk_start = {}
    for b in range(cfg.NB):
        tb = int(T[b])
        blk_start[b] = t0
        row_of_tile[t0:t0 + tb] = b
        blk = out[t0 : t0 + tb]          # [tb, BPD, SEG]
        for mrow in range(BPD):
            r = BPD * b + mrow
            if r >= cfg.S:
                continue
            D = int(pi[r])
            d = int(deg[D])
            if d == 0:
                continue
            vals = np.full(tb * SEG, cfg.ZROW, np.int64)
            vals[:d] = ls_s[starts[D] : starts[D] + d]
            blk[:, mrow, :] = vals.reshape(tb, SEG)
        t0 += tb
    # fix chunk tails: final slot of tiles CHUNK_TILES-1, 2*CHUNK_TILES-1, ...
    def row_get(blk, m, j):
        return blk[j // SEG, m, j % SEG]

    def row_swap(blk, m, j1, j2):
        a, b_ = blk[j1 // SEG, m, j1 % SEG], blk[j2 // SEG, m, j2 % SEG]
        blk[j1 // SEG, m, j1 % SEG] = b_
        blk[j2 // SEG, m, j2 % SEG] = a

    for tg in range(CHUNK_TILES - 1, total_tiles, CHUNK_TILES):
        b = int(row_of_tile[tg])
        tb = int(T[b])
        blk = out[blk_start[b] : blk_start[b] + tb]
        tl = tg - blk_start[b]
        jlast = tl * SEG + SEG - 1       # flat slot index within a row
        if blk[tl, BPD - 1, SEG - 1] >= cfg.CENTER:
            continue
        mgood = -1
        for m in range(BPD - 1, -1, -1):
            if (blk[:, m, :] >= cfg.CENTER).any():
                mgood = m
                break
        assert mgood >= 0, "no non-negative slot available for chunk tail"
        if mgood != BPD - 1:
            r1, r2 = BPD * b + mgood, BPD * b + BPD - 1
            pi[r1], pi[r2] = pi[r2], pi[r1]
            tmpv = blk[:, mgood, :].copy()
            blk[:, mgood, :] = blk[:, BPD - 1, :]
            blk[:, BPD - 1, :] = tmpv
        flat = blk[:, BPD - 1, :].reshape(-1).copy()  # contiguous copy
        j = int(np.nonzero(flat >= cfg.CENTER)[0][0])
        flat[j], flat[jlast] = flat[jlast], flat[j]
        blk[:, BPD - 1, :] = flat.reshape(tb, SEG)
    return out.reshape(total_tiles, 128)


def _wrap16(idx16):
    """[n] int16 -> [128, n/16]: idx i at partition i%16, col i//16, x8 replicas."""
    n = len(idx16)
    assert n % 16 == 0
    return np.tile(idx16.reshape(n // 16, 16).T, (8, 1)).astype(np.int16)


def _pad_idx(idx, ntot):
    out = np.full(ntot, -1, np.int64)
    out[: len(idx)] = idx
    return out


def _prep_all(inputs, cfg):
    """Host prep: per-core in_maps + the shared schedule."""
    x_user = np.asarray(inputs["x_user"], np.float32)
    x_product = np.asarray(inputs["x_product"], np.float32)
    ei = np.asarray(inputs["edge_index"]).astype(np.int64)
    u, p = ei[0], ei[1]

    metaA = [_prep_dir(u, p, c, cfg) for c in range(NCORES)]  # dst = p, src = u
    metaB = [_prep_dir(p, u, c, cfg) for c in range(NCORES)]  # dst = u, src = p

    TA = np.max([m["treq"] for m in metaA], axis=0)
    TB = np.max([m["treq"] for m in metaB], axis=0)

    def tab(x):
        t = np.zeros((cfg.N + 1, F_IN), np.float32)
        t[: cfg.N] = x
        return t

    xu_tab, xp_tab = tab(x_user), tab(x_product)

    w = {k: np.asarray(v, np.float32) for k, v in inputs.items()
         if k.startswith(("w_", "b_"))}
    shared = {
        "xu_tab": xu_tab, "xp_tab": xp_tab,
        "wu1lT": np.ascontiguousarray(w["w_u1_l"].T),
        "wu1rT": np.ascontiguousarray(w["w_u1_r"].T),
        "wp1lT": np.ascontiguousarray(w["w_p1_l"].T),
        "wp1rT": np.ascontiguousarray(w["w_p1_r"].T),
        "wu2lT": np.ascontiguousarray(w["w_u2_l"].T),
        "wu2rT": np.ascontiguousarray(w["w_u2_r"].T),
        "wp2lT": np.ascontiguousarray(w["w_p2_l"].T),
        "wp2rT": np.ascontiguousarray(w["w_p2_r"].T),
        "bu1": np.ascontiguousarray(w["b_u1"].reshape(2, 128).T),
        "bp1": np.ascontiguousarray(w["b_p1"].reshape(2, 128).T),
        "bu2": np.ascontiguousarray(w["b_u2"].reshape(CLS, 1)),
        "bp2": np.ascontiguousarray(w["b_p2"].reshape(CLS, 1)),
        "ident": np.eye(128, dtype=np.float32),
        "r8": np.repeat(np.eye(BPD, dtype=np.float32), SEG, axis=0),
    }

    in_maps = []
    for c in range(NCORES):
        d = dict(shared)
        for tag, meta, xsrc in (("A", metaA[c], x_product), ("B", metaB[c], x_user)):
            T = TA if tag == "A" else TB
            slots = _build_slots(meta, T, cfg)    # may mutate meta["pi"]
            # pad the slot array to a whole number of chunks
            nt = slots.shape[0]
            ntp = -(-nt // CHUNK_TILES) * CHUNK_TILES
            slp = np.full((ntp, 128), cfg.ZROW, np.int64)
            slp[:nt] = slots
            d[f"gidx{tag}"] = _wrap16(
                (slp.reshape(-1) - cfg.CENTER).astype(np.int16))
            pi = meta["pi"]
            d[f"unperm{tag}"] = _wrap16(
                _pad_idx(pi, cfg.SP).astype(np.int16))
            invc = np.zeros(cfg.SP, np.float32)
            invc[: cfg.S] = 1.0 / np.maximum(meta["deg"][pi], 1.0)
            d[f"invc{tag}"] = np.ascontiguousarray(
                invc.reshape(cfg.RT, 128).T)
            rows = c * cfg.S + pi
            xd = xsrc[rows]                       # [S, F] permuted dst-rows
            xdT = np.zeros((F_IN, cfg.SP), np.float32)
            xdT[:, : cfg.S] = xd.T
            d[f"xdT{tag}"] = xdT
        in_maps.append(d)

    return in_maps, TA, TB, metaA, metaB


# ---------------- device program ----------------

def _build_nc(cfg, TA, TB, local_mode=False):
    import concourse.bacc as bacc
    import concourse.mybir as mybir
    from concourse.tile import TileContext

    f32, i16 = mybir.dt.float32, mybir.dt.int16
    AF = mybir.ActivationFunctionType
    ALU = mybir.AluOpType

    nc = bacc.Bacc(None, target_bir_lowering=False, num_devices=NCORES,
                   dynamic_dma_scratch_size=49152, num_swdge_queues=1)

    S, SP, RT, NB, CENTER = cfg.S, cfg.SP, cfg.RT, cfg.NB, cfg.CENTER

    ntA = _n_tiles(TA)
    ntB = _n_tiles(TB)

    def colsA():
        return -(-ntA // CHUNK_TILES) * CHUNK_TILES * 8
    def colsB():
        return -(-ntB // CHUNK_TILES) * CHUNK_TILES * 8

    # ---- DRAM declarations ----
    t_xu = nc.dram_tensor("xu_tab", [cfg.N + 1, F_IN], f32, kind="ExternalInput")
    t_xp = nc.dram_tensor("xp_tab", [cfg.N + 1, F_IN], f32, kind="ExternalInput")
    tw = {}
    for k in ["wu1lT", "wu1rT", "wp1lT", "wp1rT"]:
        tw[k] = nc.dram_tensor(k, [F_IN, HID], f32, kind="ExternalInput")
    for k in ["wu2lT", "wu2rT", "wp2lT", "wp2rT"]:
        tw[k] = nc.dram_tensor(k, [HID, CLS], f32, kind="ExternalInput")
    for k in ["bu1", "bp1"]:
        tw[k] = nc.dram_tensor(k, [128, 2], f32, kind="ExternalInput")
    for k in ["bu2", "bp2"]:
        tw[k] = nc.dram_tensor(k, [CLS, 1], f32, kind="ExternalInput")
    t_ident = nc.dram_tensor("ident", [128, 128], f32, kind="ExternalInput")
    t_r8 = nc.dram_tensor("r8", [128, BPD], f32, kind="ExternalInput")
    t_gidxA = nc.dram_tensor("gidxA", [128, colsA()], i16, kind="ExternalInput")
    t_gidxB = nc.dram_tensor("gidxB", [128, colsB()], i16, kind="ExternalInput")
    t_unpA = nc.dram_tensor("unpermA", [128, SP // 16], i16, kind="ExternalInput")
    t_unpB = nc.dram_tensor("unpermB", [128, SP // 16], i16, kind="ExternalInput")
    t_invcA = nc.dram_tensor("invcA", [128, RT], f32, kind="ExternalInput")
    t_invcB = nc.dram_tensor("invcB", [128, RT], f32, kind="ExternalInput")
    t_xdTA = nc.dram_tensor("xdTA", [F_IN, SP], f32, kind="ExternalInput")
    t_xdTB = nc.dram_tensor("xdTB", [F_IN, SP], f32, kind="ExternalInput")

    t_xu2 = nc.dram_tensor("xu2", [SP, CLS], f32, kind="ExternalOutput")
    t_xp2 = nc.dram_tensor("xp2", [SP, CLS], f32, kind="ExternalOutput")

    st_zu = nc.dram_tensor("zu_stage", [SP, CLS], f32)
    st_zp = nc.dram_tensor("zp_stage", [SP, CLS], f32)
    st_r2A = nc.dram_tensor("r2A_stage", [SP, CLS], f32)
    st_r2B = nc.dram_tensor("r2B_stage", [SP, CLS], f32)
    aspace = "Local" if local_mode else "Shared"
    t_zuf = nc.dram_tensor("zu_full", [cfg.N + 1, CLS], f32, addr_space=aspace)
    t_zpf = nc.dram_tensor("zp_full", [cfg.N + 1, CLS], f32, addr_space=aspace)

    with TileContext(nc) as tc:
        # ---- persistent SBUF ----
        with tc.tile_pool(name="persist", bufs=1) as pp:
            sb_ident = pp.tile([128, 128], f32)
            sb_r8 = pp.tile([128, BPD], f32)
            sb_gidxA = pp.tile([128, colsA()], i16)
            sb_gidxB = pp.tile([128, colsB()], i16)
            sb_w = {}
            for k in ["wu1lT", "wu1rT", "wp1lT", "wp1rT"]:
                sb_w[k] = pp.tile([F_IN, HID], f32, tag=k, name=k)
            for k in ["wu2lT", "wu2rT", "wp2lT", "wp2rT"]:
                sb_w[k] = pp.tile([128, 2, CLS], f32, tag=k, name=k)
            for k in ["bu1", "bp1"]:
                sb_w[k] = pp.tile([128, 2], f32, tag=k, name=k)
            b2 = {}
            for k in ["bu2", "bp2"]:
                b2[k] = pp.tile([128, 1], f32, tag=k, name=k)
            sb_invcA = pp.tile([128, RT], f32)
            sb_invcB = pp.tile([128, RT], f32)
            sb_unpA = pp.tile([128, SP // 16], i16)
            sb_unpB = pp.tile([128, SP // 16], i16)

            nc.sync.dma_start(out=sb_ident[:], in_=t_ident[:])
            nc.sync.dma_start(out=sb_r8[:], in_=t_r8[:])
            nc.sync.dma_start(out=sb_gidxA[:], in_=t_gidxA[:])
            nc.sync.dma_start(out=sb_gidxB[:], in_=t_gidxB[:])
            for k, t in tw.items():
                if k in ("bu2", "bp2"):
                    nc.sync.dma_start(out=b2[k][64:64 + CLS, :], in_=t[:])
                elif k in ("wu2lT", "wu2rT", "wp2lT", "wp2rT"):
                    nc.sync.dma_start(
                        out=sb_w[k][:],
                        in_=t.rearrange("(k p) c -> p k c", p=128)[:])
                else:
                    nc.sync.dma_start(out=sb_w[k][:], in_=t[:])
            nc.sync.dma_start(out=sb_invcA[:], in_=t_invcA[:])
            nc.sync.dma_start(out=sb_invcB[:], in_=t_invcB[:])
            nc.sync.dma_start(out=sb_unpA[:], in_=t_unpA[:])
            nc.sync.dma_start(out=sb_unpB[:], in_=t_unpB[:])

            # zero the scatter-target stages (+ z_full zero row)
            with tc.tile_pool(name="zpool", bufs=1) as zp:
                zt = zp.tile([128, RT, CLS], f32)
                nc.vector.memset(zt[:], 0.0)
                for st in (st_zu, st_zp, st_r2A, st_r2B, t_xu2, t_xp2):
                    nc.sync.dma_start(
                        out=st.rearrange("(c p) f -> p c f", p=128)[:], in_=zt[:])
                nc.sync.dma_start(out=t_zuf[cfg.N:cfg.N + 1, :], in_=zt[0:1, 0, :])
                nc.sync.dma_start(out=t_zpf[cfg.N:cfg.N + 1, :], in_=zt[0:1, 0, :])

            # ================= aggregation pass emitter =================
            def agg_pass(gidx_sb, T, table_ap, elem, agg_sb, label):
                ntiles = _n_tiles(T)
                with tc.tile_pool(name=f"msg{label}", bufs=4) as mp, \
                     tc.tile_pool(name=f"aggps{label}", bufs=8, space="PSUM") as ap:
                    msgs = {}

                    def chunk_of(tg):
                        ch = tg // CHUNK_TILES
                        if ch not in msgs:
                            t0c = ch * CHUNK_TILES
                            ct = min(CHUNK_TILES, ntiles - t0c)
                            m = mp.tile([128, CHUNK_TILES, elem], f32,
                                        tag="msg", name=f"msg{label}_{ch}")
                            nc.gpsimd.dma_gather(
                                m[:, :ct, :], table_ap,
                                gidx_sb[:, 8 * t0c:8 * t0c + 8 * ct],
                                ct * 128, ct * 128, elem)
                            msgs[ch] = m
                        return msgs[ch]

                    tg = 0
                    pb = 128 // BPD
                    for b in range(cfg.NB):
                        ps = ap.tile([BPD, elem], f32, tag="ps",
                                     name=f"ps{label}_{b}")
                        for k in range(int(T[b])):
                            m = chunk_of(tg)
                            nc.tensor.matmul(
                                ps[:], sb_r8[:], m[:, tg % CHUNK_TILES, :],
                                start=(k == 0), stop=(k == int(T[b]) - 1))
                            tg += 1
                        nc.vector.tensor_copy(
                            agg_sb[BPD * (b % pb):BPD * (b % pb) + BPD,
                                   b // pb, :], ps[:])

            # ================= phase-3 emitter (per direction) =================
            # consumes agg (row-major, permuted), xdT; produces z + r2_other
            def phase3(agg_sb, xdT_t, invc_sb, wl, wr, b1k, w2l, w2r_o, b2_o,
                       st_z, st_r2o, unp_sb, label):
                with tc.tile_pool(name=f"p3{label}", bufs=1) as p3, \
                     tc.tile_pool(name=f"p3w{label}", bufs=2) as p3w, \
                     tc.tile_pool(name=f"psT{label}", bufs=2, space="PSUM") as psT, \
                     tc.tile_pool(name=f"psG{label}", bufs=4, space="PSUM") as psG, \
                     tc.tile_pool(name=f"psZ{label}", bufs=2, space="PSUM") as psZ:
                    xdT = p3.tile([F_IN, SP], f32, tag="xdT")
                    nc.sync.dma_start(out=xdT[:], in_=xdT_t[:])
                    x1T = p3.tile([128, 2, SP], f32, tag="x1T")
                    zrows = p3.tile([128, RT, CLS], f32, tag="zrows")
                    r2rows = p3.tile([128, RT, CLS], f32, tag="r2rows")
                    ngr = -(-RT // 4)
                    for g in range(ngr):
                        jj0 = 4 * g
                        njj = min(4, RT - jj0)
                        rg = njj * 128
                        aT = p3w.tile([128, 512], f32, tag="aT")
                        for q in range(njj):
                            mt = p3w.tile([128, 128], f32, tag="mt")
                            nc.vector.tensor_scalar_mul(
                                mt[:], agg_sb[:, jj0 + q, :],
                                invc_sb[:, jj0 + q:jj0 + q + 1])
                            pt = psT.tile([128, 128], f32, tag="pt")
                            nc.tensor.transpose(pt[:], mt[:], sb_ident[:])
                            nc.vector.tensor_copy(
                                aT[:, 128 * q:128 * q + 128], pt[:])
                        c0 = 512 * g
                        for h in range(2):
                            po = psG.tile([128, 512], f32, tag="po")
                            nc.tensor.matmul(
                                po[:, :rg], wl[:, 128 * h:128 * h + 128],
                                aT[:, :rg], start=True, stop=False)
                            nc.tensor.matmul(
                                po[:, :rg], wr[:, 128 * h:128 * h + 128],
                                xdT[:, c0:c0 + rg], start=False, stop=True)
                            nc.scalar.activation(
                                x1T[:, h, c0:c0 + rg], po[:, :rg], AF.Relu,
                                bias=b1k[:, h:h + 1])
                        pz = psZ.tile([128, 512], f32, tag="pz")
                        for h in range(2):
                            nc.tensor.matmul(
                                pz[0:CLS, :rg], w2l[:, h, :],
                                x1T[:, h, c0:c0 + rg],
                                start=(h == 0), stop=(h == 1))
                        for h in range(2):
                            nc.tensor.matmul(
                                pz[64:64 + CLS, :rg], w2r_o[:, h, :],
                                x1T[:, h, c0:c0 + rg],
                                start=(h == 0), stop=(h == 1))
                        zr2 = p3w.tile([128, 512], f32, tag="zr2")
                        nc.vector.tensor_copy(zr2[0:CLS, :rg], pz[0:CLS, :rg])
                        nc.vector.tensor_scalar_add(
                            zr2[64:64 + CLS, :rg], pz[64:64 + CLS, :rg],
                            b2_o[64:64 + CLS, 0:1])
                        for q in range(njj):
                            pb = psT.tile([128, 128], f32, tag="pt")
                            nc.tensor.transpose(
                                pb[:, :], zr2[:, 128 * q:128 * q + 128],
                                sb_ident[:])
                            nc.vector.tensor_copy(
                                zrows[:, jj0 + q, :], pb[:, 0:CLS])
                            nc.vector.tensor_copy(
                                r2rows[:, jj0 + q, :], pb[:, 64:64 + CLS])
                    for k0 in range(0, SP, SCAT_CHUNK):
                        nv = min(SCAT_CHUNK, S - k0)
                        if nv <= 0:
                            break
                        kt = min(SCAT_CHUNK, SP - k0) // 128
                        sl = slice(k0 // 128, k0 // 128 + kt)
                        ic = slice(k0 // 16, (k0 + kt * 128) // 16)
                        nc.gpsimd.dma_scatter_add(
                            st_z[:], zrows[:, sl, :], unp_sb[:, ic],
                            kt * 128, nv, CLS)
                        nc.gpsimd.dma_scatter_add(
                            st_r2o[:], r2rows[:, sl, :], unp_sb[:, ic],
                            kt * 128, nv, CLS)

            # ================= phase-7 emitter =================
            def phase7(agg2_sb, invc_sb, st_r2, unp_sb, t_out, label):
                with tc.tile_pool(name=f"p7{label}", bufs=1) as p7:
                    r2r = p7.tile([128, RT, CLS], f32, tag="r2r")
                    GCH = 1024
                    for k0 in range(0, SP, GCH):
                        nv = min(GCH, S - k0)
                        if nv <= 0:
                            break
                        kt = min(GCH, SP - k0) // 128
                        nc.gpsimd.dma_gather(
                            r2r[:, k0 // 128:k0 // 128 + kt, :], st_r2[:],
                            unp_sb[:, k0 // 16:(k0 + 128 * kt) // 16],
                            kt * 128, min(nv, kt * 128), CLS)
                    outt = p7.tile([128, RT, CLS], f32, tag="outt")
                    for q in range(RT):
                        tmp = p7.tile([128, CLS], f32, tag="tmp")
                        nc.vector.tensor_scalar_mul(
                            tmp[:], agg2_sb[:, q, :], invc_sb[:, q:q + 1])
                        nc.vector.tensor_tensor(
                            out=outt[:, q, :], in0=tmp[:], in1=r2r[:, q, :],
                            op=ALU.add)
                    for k0 in range(0, SP, SCAT_CHUNK):
                        nv = min(SCAT_CHUNK, S - k0)
                        if nv <= 0:
                            break
                        kt = min(SCAT_CHUNK, SP - k0) // 128
                        nc.gpsimd.dma_scatter_add(
                            t_out[:], outt[:, k0 // 128:k0 // 128 + kt, :],
                            unp_sb[:, k0 // 16:(k0 + kt * 128) // 16],
                            kt * 128, nv, CLS)

            # ================= emit the whole program =================
            import os as _os
            PARTS = set((_os.environ.get("KERNEL_PARTS") or
                         "agg1,p3,cc,agg2,p7").split(","))
            with tc.tile_pool(name="aggAp", bufs=1) as aggApool:
                aggA = aggApool.tile([128, RT, F_IN], f32)
                if "agg1" in PARTS:
                    agg_pass(sb_gidxA, TA, t_xu[CENTER:, :], F_IN, aggA, "A")
                if "p3" in PARTS:
                    phase3(aggA, t_xdTA, sb_invcA, sb_w["wu1lT"], sb_w["wu1rT"],
                           sb_w["bu1"], sb_w["wu2lT"], sb_w["wp2rT"], b2["bp2"],
                           st_zu, st_r2B, sb_unpA, "A")
            with tc.tile_pool(name="aggBp", bufs=1) as aggBpool:
                aggB = aggBpool.tile([128, RT, F_IN], f32)
                if "agg1" in PARTS:
                    agg_pass(sb_gidxB, TB, t_xp[CENTER:, :], F_IN, aggB, "B")
                if "p3" in PARTS:
                    phase3(aggB, t_xdTB, sb_invcB, sb_w["wp1lT"], sb_w["wp1rT"],
                           sb_w["bp1"], sb_w["wp2lT"], sb_w["wu2rT"], b2["bu2"],
                           st_zp, st_r2A, sb_unpB, "B")

            if "cc" not in PARTS:
                pass
            elif local_mode:
                nc.sync.dma_start(out=t_zuf[0:S, :], in_=st_zu[0:S, :])
                nc.sync.dma_start(out=t_zpf[0:S, :], in_=st_zp[0:S, :])
            else:
                nc.gpsimd.collective_compute(
                    "AllGather", mybir.AluOpType.bypass,
                    replica_groups=[list(range(NCORES))],
                    ins=[st_zu[0:S, :]], outs=[t_zuf[0:cfg.N, :]])
                nc.gpsimd.collective_compute(
                    "AllGather", mybir.AluOpType.bypass,
                    replica_groups=[list(range(NCORES))],
                    ins=[st_zp[0:S, :]], outs=[t_zpf[0:cfg.N, :]])

            with tc.tile_pool(name="agg2Ap", bufs=1) as a2p:
                agg2A = a2p.tile([128, RT, CLS], f32)
                if "agg2" in PARTS:
                    agg_pass(sb_gidxA, TA, t_zuf[CENTER:, :], CLS, agg2A, "A2")
                if "p7" in PARTS:
                    phase7(agg2A, sb_invcA, st_r2A, sb_unpA, t_xu2, "A")
            with tc.tile_pool(name="agg2Bp", bufs=1) as b2p:
                agg2B = b2p.tile([128, RT, CLS], f32)
                if "agg2" in PARTS:
                    agg_pass(sb_gidxB, TB, t_zpf[CENTER:, :], CLS, agg2B, "B2")
                if "p7" in PARTS:
                    phase7(agg2B, sb_invcB, st_r2B, sb_unpB, t_xp2, "B")

    nc.finalize()
    return nc


def build(inputs, cfg=None, local_mode=False):
    cfg = cfg or CFG()
    in_maps, TA, TB, metaA, metaB = _prep_all(inputs, cfg)
    nc = _build_nc(cfg, TA, TB, local_mode=local_mode)
    return nc, in_maps


def kernel(**inputs):
    from concourse.bass_utils import run_bass_kernel_spmd

    cfg = CFG()
    nc, in_maps = build(inputs, cfg)
    res = run_bass_kernel_spmd(nc, in_maps, list(range(NCORES)))
    xu2 = np.concatenate(
        [res.results[c]["xu2"][: cfg.S] for c in range(NCORES)], 0)
    xp2 = np.concatenate(
        [res.results[c]["xp2"][: cfg.S] for c in range(NCORES)], 0)
    return xu2, xp2



# revision 3
# speedup vs baseline: 1.5410x; 1.5410x over previous
"""Bipartite 2-layer SAGEConv GNN on 8 Trainium2 NeuronCores.

Strategy (v2):
  - Edges sharded by destination range (core c owns dst rows [S*c, S*(c+1))
    for BOTH directions). Per core+direction, dsts sorted by degree; edges
    packed into 2-slot segments, 64 dst-rows per PSUM block, variable tiles
    per block (schedule = max over cores -> one SPMD program for all cores).
  - All tables/messages/weights fp16 (PE 1 cycle/row vs 4 for fp32).
  - Segment-sum TRANSPOSED: out[feat, dst] = msg_tile^T @ R8inv where
    R8inv is the one-hot segment matrix pre-scaled by 1/deg on the host.
    Output free size = 64 (dsts) and arrives pre-transposed for the GEMMs,
    so the mean pass and all input-side transposes disappear.
  - Message gather: dma_gather from fp16 tables (256B rows), CENTERED base
    (idx int16, idx = row - N/2). Gather calls have variable tile counts:
    call boundaries are chosen so every call's last index can be made
    non-negative by an order-only swap within its dst row (the SWDGE ucode
    trims trailing negatives). No dst-permutation mutation needed.
  - Layer 2 transform-first: z = x1 @ w2l.T (64 wide) is stored in a
    combined fp16 table zc[i] = [zu | zp] (256B rows) in DEGREE-PERMUTED
    order (host remaps layer-2 gather indices), so the z store is a plain
    strided DMA. One AllGather of zc.
  - lin_r terms (r2) and layer-2 means are emitted in transposed, permuted
    form; the host un-permutes/adds/biases them for free in numpy.
  - No dma_scatter_add anywhere.
"""
import sys
import numpy as np

sys.path.insert(0, "/opt/trn_rl_repo")

# ---------------- problem dims (hardcoded for the harness) ----------------
N = 50000
E = 800000
F_IN = 128
HID = 256
CLS = 64
NCORES = 8

SEG = 2            # slots per segment (one dst's edges per tile-row)
BPD = 64           # dsts per psum block
CHUNK_TILES = 12   # max tiles per gather call (1536 idx; SWDGE ring = 3072)
GRP = 512          # dst columns per phase-3 GEMM group


class CFG:
    def __init__(self, n=N, e=E, center=None):
        self.N = n
        self.E = e
        self.S = n // NCORES          # dst rows per core
        self.CENTER = n // 2 if center is None else center
        self.ZROW = n                 # zero row index of gather tables
        self.NB = -(-self.S // BPD)   # blocks per direction
        self.RT = -(-self.S // 128)   # 128-row tiles of the slice
        self.SP = self.NB * BPD       # padded rows


# ---------------- host-side edge scheduling ----------------

def _prep_dir(src_g, dst_g, c, cfg):
    """Per-core, per-direction metadata. pi is frozen (pure degree sort)."""
    lo = c * cfg.S
    m = (dst_g >= lo) & (dst_g < lo + cfg.S)
    ls = src_g[m].astype(np.int64)
    ld = (dst_g[m] - lo).astype(np.int64)
    deg = np.bincount(ld, minlength=cfg.S)
    pi = np.argsort(-deg, kind="stable").astype(np.int64)
    order = np.argsort(ld, kind="stable")
    ls_s = ls[order]
    starts = np.zeros(cfg.S + 1, np.int64)
    starts[1:] = np.cumsum(deg)
    degp = np.zeros(cfg.NB * BPD, np.int64)
    degp[: cfg.S] = deg[pi]
    treq = np.maximum(
        1, -(-degp.reshape(cfg.NB, BPD).max(1) // SEG)
    ).astype(np.int64)
    return dict(pi=pi, deg=deg, starts=starts, ls_s=ls_s, degp=degp, treq=treq)


def _build_slots(meta, T, cfg):
    """Slot array [ntiles, 128] of src node ids (ZROW for dummies)."""
    pi, deg, starts, ls_s = meta["pi"], meta["deg"], meta["starts"], meta["ls_s"]
    total_tiles = int(T.sum())
    out = np.full((total_tiles, BPD, SEG), cfg.ZROW, np.int64)
    t0 = 0
    for b in range(cfg.NB):
        tb = int(T[b])
        blk = out[t0: t0 + tb]
        for mrow in range(BPD):
            r = BPD * b + mrow
            if r >= cfg.S:
                continue
            D = int(pi[r])
            d = int(deg[D])
            if d == 0:
                continue
            vals = np.full(tb * SEG, cfg.ZROW, np.int64)
            vals[:d] = ls_s[starts[D]: starts[D] + d]
            blk[:, mrow, :] = vals.reshape(tb, SEG)
        t0 += tb
    return out.reshape(total_tiles, 128)


def _tile_blocks(T):
    """block index per tile + block start tile."""
    nt = int(T.sum())
    row_of_tile = np.zeros(nt, np.int64)
    blk_start = np.zeros(len(T), np.int64)
    t0 = 0
    for b, tb in enumerate(T):
        blk_start[b] = t0
        row_of_tile[t0: t0 + int(tb)] = b
        t0 += int(tb)
    return row_of_tile, blk_start


def _fix_tails(arrs, T, cfg):
    """Choose shared gather-call boundaries and fix each per-core slot array
    so every call's final slot value is >= CENTER (order-only swaps within
    the dst row at partition 127).

    arrs: list (per core) of [nt, 128] slot-value arrays, mutated in place.
    Returns list of (t0, ct) gather calls.
    """
    nt = arrs[0].shape[0]
    row_of_tile, blk_start = _tile_blocks(T)
    reserved = [dict() for _ in arrs]   # core -> {block: set(stream pos)}

    def stream(a, b):
        """Flat slot stream of partition-127's dst row in block b."""
        tb = int(T[b])
        s0 = int(blk_start[b])
        return a[s0: s0 + tb, (BPD - 1) * SEG:].reshape(-1)

    def fix_one(a, res, b, jpos, apply):
        st = stream(a, b)
        r = res.setdefault(b, set())
        if st[jpos] >= cfg.CENTER:
            if apply:
                r.add(jpos)
            return True
        cand = np.nonzero(st >= cfg.CENTER)[0]
        cand = [j for j in cand if j not in r and j != jpos]
        if not cand:
            return False
        if apply:
            j = int(cand[0])
            tb = int(T[b])
            s0 = int(blk_start[b])
            view = a[s0: s0 + tb, (BPD - 1) * SEG:].reshape(-1)
            view[jpos], view[j] = view[j], view[jpos]
            a[s0: s0 + tb, (BPD - 1) * SEG:] = view.reshape(tb, SEG)
            r.add(jpos)
        return True

    calls = []
    t0 = 0
    while t0 < nt:
        ct = min(CHUNK_TILES, nt - t0)
        chosen = None
        for bnd in range(t0 + ct, t0, -1):
            tb1 = bnd - 1
            b = int(row_of_tile[tb1])
            jpos = int(tb1 - blk_start[b]) * SEG + (SEG - 1)
            if all(fix_one(arrs[c], reserved[c], b, jpos, False)
                   for c in range(len(arrs))):
                chosen = bnd
                for c in range(len(arrs)):
                    fix_one(arrs[c], reserved[c], b, jpos, True)
                break
        assert chosen is not None, "no fixable gather boundary in window"
        calls.append((t0, chosen - t0))
        t0 = chosen
    return calls


def _wrap16(idx16):
    """[n] int16 -> [128, n/16]: idx i at partition i%16, col i//16, x8."""
    n = len(idx16)
    assert n % 16 == 0
    return np.tile(idx16.reshape(n // 16, 16).T, (8, 1)).astype(np.int16)


def _prep_all(inputs, cfg):
    f16 = np.float16
    x_user = np.asarray(inputs["x_user"], np.float32)
    x_product = np.asarray(inputs["x_product"], np.float32)
    ei = np.asarray(inputs["edge_index"]).astype(np.int64)
    u, p = ei[0], ei[1]

    metaA = [_prep_dir(u, p, c, cfg) for c in range(NCORES)]  # dst=p, src=u
    metaB = [_prep_dir(p, u, c, cfg) for c in range(NCORES)]  # dst=u, src=p

    TA = np.max([m["treq"] for m in metaA], axis=0)
    TB = np.max([m["treq"] for m in metaB], axis=0)

    # layer-1 slot arrays + call schedules
    sl1A = [_build_slots(metaA[c], TA, cfg) for c in range(NCORES)]
    sl1B = [_build_slots(metaB[c], TB, cfg) for c in range(NCORES)]
    c1A = _fix_tails(sl1A, TA, cfg)
    c1B = _fix_tails(sl1B, TB, cfg)

    # z-table position maps (degree-permuted layout, global)
    PA = np.empty(cfg.N + 1, np.int64)
    PB = np.empty(cfg.N + 1, np.int64)
    ar = np.arange(cfg.S, dtype=np.int64)
    for c in range(NCORES):
        PA[c * cfg.S + metaA[c]["pi"]] = c * cfg.S + ar
        PB[c * cfg.S + metaB[c]["pi"]] = c * cfg.S + ar
    PA[cfg.N] = cfg.N
    PB[cfg.N] = cfg.N

    sl2A = [PA[a] for a in sl1A]
    sl2B = [PB[a] for a in sl1B]
    c2A = _fix_tails(sl2A, TA, cfg)
    c2B = _fix_tails(sl2B, TB, cfg)

    def tab(x):
        t = np.zeros((cfg.N + 1, F_IN), f16)
        t[: cfg.N] = x.astype(f16)
        return t

    w = {k: np.asarray(v, np.float32) for k, v in inputs.items()
         if k.startswith(("w_", "b_"))}

    r8 = np.repeat(np.eye(BPD, dtype=np.float32), SEG, axis=0)  # [128, BPD]

    def r8inv(meta):
        invc = 1.0 / np.maximum(meta["degp"], 1.0)               # [NB*BPD]
        m = r8[:, None, :] * invc.reshape(cfg.NB, BPD)[None, :, :]
        return np.ascontiguousarray(
            m.reshape(128, cfg.NB * BPD).astype(f16))

    def w2stack(wz, wr):
        # [128, 2, 128]: cols 0:64 = wz.T chunk, 64:128 = wr.T chunk
        s = np.zeros((128, 2, 128), f16)
        for h in range(2):
            s[:, h, 0:CLS] = wz.T[128 * h: 128 * (h + 1), :]
            s[:, h, CLS:128] = wr.T[128 * h: 128 * (h + 1), :]
        return np.ascontiguousarray(s.reshape(128, 256))

    shared = {
        "xu_tab": tab(x_user), "xp_tab": tab(x_product),
        "wu1l": np.ascontiguousarray(w["w_u1_l"].T.astype(f16)),
        "wu1r": np.ascontiguousarray(w["w_u1_r"].T.astype(f16)),
        "wp1l": np.ascontiguousarray(w["w_p1_l"].T.astype(f16)),
        "wp1r": np.ascontiguousarray(w["w_p1_r"].T.astype(f16)),
        "w2A": w2stack(w["w_u2_l"], w["w_p2_r"]),   # z=zu, r2 -> xp2
        "w2B": w2stack(w["w_p2_l"], w["w_u2_r"]),   # z=zp, r2 -> xu2
        "bu1": np.ascontiguousarray(w["b_u1"].reshape(2, 128).T.astype(np.float32)),
        "bp1": np.ascontiguousarray(w["b_p1"].reshape(2, 128).T.astype(np.float32)),
        "ident": np.eye(128, dtype=f16),
    }

    in_maps = []
    for c in range(NCORES):
        d = dict(shared)
        for tag, meta, sl1, sl2, xsrc in (
                ("A", metaA[c], sl1A[c], sl2A[c], x_product),
                ("B", metaB[c], sl1B[c], sl2B[c], x_user)):
            d[f"g1{tag}"] = _wrap16(
                (sl1.reshape(-1) - cfg.CENTER).astype(np.int16))
            d[f"g2{tag}"] = _wrap16(
                (sl2.reshape(-1) - cfg.CENTER).astype(np.int16))
            d[f"r8i{tag}"] = r8inv(meta)
            rows = c * cfg.S + meta["pi"]
            xdT = np.zeros((F_IN, cfg.SP), f16)
            xdT[:, : cfg.S] = xsrc[rows].T.astype(f16)
            d[f"xdT{tag}"] = np.ascontiguousarray(xdT)
        in_maps.append(d)

    host_ctx = {
        "piA": [metaA[c]["pi"] for c in range(NCORES)],
        "piB": [metaB[c]["pi"] for c in range(NCORES)],
        "b_u2": w["b_u2"], "b_p2": w["b_p2"],
    }
    return in_maps, (TA, c1A, c2A), (TB, c1B, c2B), host_ctx


# ---------------- device program ----------------

def _build_nc(cfg, schedA, schedB, local_mode=False):
    import concourse.bacc as bacc
    import concourse.mybir as mybir
    from concourse.tile import TileContext

    f32, f16, i16 = mybir.dt.float32, mybir.dt.float16, mybir.dt.int16
    AF = mybir.ActivationFunctionType

    nc = bacc.Bacc(None, target_bir_lowering=False, num_devices=NCORES,
                   dynamic_dma_scratch_size=49152, num_swdge_queues=1)

    S, SP, NB, CENTER = cfg.S, cfg.SP, cfg.NB, cfg.CENTER
    TA, c1A, c2A = schedA
    TB, c1B, c2B = schedB
    ntA, ntB = int(TA.sum()), int(TB.sum())
    GMAX = max(ntA, ntB) * 8

    # ---- DRAM declarations ----
    t_xu = nc.dram_tensor("xu_tab", [cfg.N + 1, F_IN], f16, kind="ExternalInput")
    t_xp = nc.dram_tensor("xp_tab", [cfg.N + 1, F_IN], f16, kind="ExternalInput")
    tw = {}
    for k in ["wu1l", "wu1r", "wp1l", "wp1r", "w2A", "w2B"]:
        tw[k] = nc.dram_tensor(k, [128, 256], f16, kind="ExternalInput")
    for k in ["bu1", "bp1"]:
        tw[k] = nc.dram_tensor(k, [128, 2], f32, kind="ExternalInput")
    t_ident = nc.dram_tensor("ident", [128, 128], f16, kind="ExternalInput")
    t_g = {}
    for k, nt in (("g1A", ntA), ("g1B", ntB), ("g2A", ntA), ("g2B", ntB)):
        t_g[k] = nc.dram_tensor(k, [128, nt * 8], i16, kind="ExternalInput")
    t_r8iA = nc.dram_tensor("r8iA", [128, SP], f16, kind="ExternalInput")
    t_r8iB = nc.dram_tensor("r8iB", [128, SP], f16, kind="ExternalInput")
    t_xdTA = nc.dram_tensor("xdTA", [F_IN, SP], f16, kind="ExternalInput")
    t_xdTB = nc.dram_tensor("xdTB", [F_IN, SP], f16, kind="ExternalInput")

    outs = {k: nc.dram_tensor(k, [CLS, SP], f16, kind="ExternalOutput")
            for k in ["mu", "ru", "mp", "rp"]}

    st_zc = nc.dram_tensor("zc_stage", [SP, 128], f16)
    aspace = "Local" if local_mode else "Shared"
    t_zcf = nc.dram_tensor("zc_full", [cfg.N + 1, 128], f16, addr_space=aspace)

    with TileContext(nc) as tc:
        with tc.tile_pool(name="persist", bufs=1) as pp, \
             tc.tile_pool(name="gidx", bufs=2) as gp, \
             tc.tile_pool(name="big", bufs=3) as bigp:
            sb_ident = pp.tile([128, 128], f16)
            sb_w = {}
            for k in ["wu1l", "wu1r", "wp1l", "wp1r", "w2A", "w2B"]:
                sb_w[k] = pp.tile([128, 256], f16, tag=k, name=k)
            for k in ["bu1", "bp1"]:
                sb_w[k] = pp.tile([128, 2], f32, tag=k, name=k)
            sb_r8iA = pp.tile([128, SP], f16)
            sb_r8iB = pp.tile([128, SP], f16)

            sb_g1A = gp.tile([128, GMAX], i16, tag="gidx", name="g1A")
            sb_g1B = gp.tile([128, GMAX], i16, tag="gidx", name="g1B")
            nc.sync.dma_start(out=sb_g1A[:, : ntA * 8], in_=t_g["g1A"][:])
            nc.sync.dma_start(out=sb_g1B[:, : ntB * 8], in_=t_g["g1B"][:])
            nc.sync.dma_start(out=sb_ident[:], in_=t_ident[:])
            for k, t in tw.items():
                nc.sync.dma_start(out=sb_w[k][:], in_=t[:])
            nc.sync.dma_start(out=sb_r8iA[:], in_=t_r8iA[:])
            nc.sync.dma_start(out=sb_r8iB[:], in_=t_r8iB[:])

            # zero row of the z table
            with tc.tile_pool(name="zrow", bufs=1) as zp:
                zt = zp.tile([1, 128], f16)
                nc.vector.memset(zt[:], 0.0)
                nc.sync.dma_start(out=t_zcf[cfg.N: cfg.N + 1, :], in_=zt[:])

            # ================= aggregation pass emitter =================
            def agg_pass(gidx_sb, T, calls, table_ap, r8i_sb, out_sb,
                         out_parts, lcol, label):
                row_of_tile, blk_start = _tile_blocks(T)
                call_of_tile = np.zeros(int(T.sum()), np.int64)
                for k, (t0, ct) in enumerate(calls):
                    call_of_tile[t0: t0 + ct] = k
                with tc.tile_pool(name=f"msg{label}", bufs=8) as mp, \
                     tc.tile_pool(name=f"agg{label}", bufs=4,
                                  space="PSUM") as ap:
                    msgs = {}

                    def chunk_of(tg):
                        k = int(call_of_tile[tg])
                        if k not in msgs:
                            t0, ct = calls[k]
                            m = mp.tile([128, CHUNK_TILES, F_IN], f16,
                                        tag="msg", name=f"msg{label}_{k}")
                            nc.gpsimd.dma_gather(
                                m[:, :ct, :], table_ap,
                                gidx_sb[:, 8 * t0: 8 * t0 + 8 * ct],
                                ct * 128, ct * 128, F_IN)
                            msgs[k] = m
                        return msgs[k], calls[k][0]

                    tg = 0
                    for b in range(cfg.NB):
                        ps = ap.tile([out_parts, BPD], f32, tag="ps",
                                     name=f"ps{label}_{b}")
                        nt_b = int(T[b])
                        for k in range(nt_b):
                            m, t0 = chunk_of(tg)
                            if lcol is None:
                                lhsT = m[:, tg - t0, :]
                            else:
                                lhsT = m[:, tg - t0,
                                         lcol * CLS: (lcol + 1) * CLS]
                            nc.tensor.matmul(
                                ps[:], lhsT, r8i_sb[:, b * BPD: (b + 1) * BPD],
                                start=(k == 0), stop=(k == nt_b - 1))
                            tg += 1
                        nc.vector.tensor_copy(
                            out_sb[0:out_parts, b * BPD: (b + 1) * BPD], ps[:])

            # ================= phase-3 emitter (per direction) =================
            def phase3(meanT, xdT_t, wl, wr, b1, w2s, zhalf, t_r2, label):
                with tc.tile_pool(name=f"p3s{label}", bufs=2) as p3s, \
                     tc.tile_pool(name=f"x1{label}", bufs=2) as x1p, \
                     tc.tile_pool(name=f"po{label}", bufs=2, space="PSUM") as pop, \
                     tc.tile_pool(name=f"pz{label}", bufs=2, space="PSUM") as pzp, \
                     tc.tile_pool(name=f"pt{label}", bufs=2, space="PSUM") as ptp:
                    xdT = bigp.tile([128, SP], f16, tag="big",
                                    name=f"xdT{label}")
                    nc.sync.dma_start(out=xdT[:], in_=xdT_t[:])
                    ngr = -(-SP // GRP)
                    for g in range(ngr):
                        c0 = GRP * g
                        rg = min(GRP, SP - c0)
                        x1g = x1p.tile([128, 2, GRP], f16, tag="x1")
                        po = pop.tile([128, GRP], f32, tag="po")
                        for h in range(2):
                            nc.tensor.matmul(
                                po[:, :rg], wl[:, 128 * h: 128 * (h + 1)],
                                meanT[:, c0: c0 + rg], start=True, stop=False)
                            nc.tensor.matmul(
                                po[:, :rg], wr[:, 128 * h: 128 * (h + 1)],
                                xdT[:, c0: c0 + rg], start=False, stop=True)
                            nc.scalar.activation(
                                x1g[:, h, :rg], po[:, :rg], AF.Relu,
                                bias=b1[:, h: h + 1])
                        pz = pzp.tile([128, GRP], f32, tag="pz")
                        for h in range(2):
                            nc.tensor.matmul(
                                pz[:, :rg], w2s[:, 128 * h: 128 * (h + 1)],
                                x1g[:, h, :rg], start=(h == 0), stop=(h == 1))
                        zr = p3s.tile([128, GRP], f16, tag="zr")
                        nc.vector.tensor_copy(zr[:, :rg], pz[:, :rg])
                        # r2 rows (partitions 64:128) -> transposed output
                        nc.sync.dma_start(
                            out=t_r2[:, c0: c0 + rg], in_=zr[CLS:128, :rg])
                        # z rows (partitions 0:64) -> transpose -> zc table
                        for q in range(-(-rg // 128)):
                            cw = min(128, rg - 128 * q)
                            pt = ptp.tile([128, CLS], f16, tag="pt")
                            nc.tensor.transpose(
                                pt[0:cw, :], zr[0:CLS, 128 * q: 128 * q + cw],
                                sb_ident[0:CLS, 0:CLS])
                            zt = p3s.tile([128, CLS], f16, tag="zt")
                            nc.vector.tensor_copy(zt[0:cw, :], pt[0:cw, :])
                            r0 = c0 + 128 * q
                            nc.sync.dma_start(
                                out=st_zc[r0: r0 + cw,
                                          zhalf * CLS: (zhalf + 1) * CLS],
                                in_=zt[0:cw, :])

            # ================= emit the whole program =================
            import os as _os
            PARTS = set((_os.environ.get("KERNEL_PARTS") or
                         "agg1,p3,cc,agg2").split(","))

            meanTA = bigp.tile([128, SP], f16, tag="big", name="meanTA")
            if "agg1" in PARTS:
                agg_pass(sb_g1A, TA, c1A, t_xu[CENTER:, :], sb_r8iA,
                         meanTA, 128, None, "A")
            if "p3" in PARTS:
                phase3(meanTA, t_xdTA, sb_w["wu1l"], sb_w["wu1r"],
                       sb_w["bu1"], sb_w["w2A"], 0, outs["rp"], "A")
            meanTB = bigp.tile([128, SP], f16, tag="big", name="meanTB")
            if "agg1" in PARTS:
                agg_pass(sb_g1B, TB, c1B, t_xp[CENTER:, :], sb_r8iB,
                         meanTB, 128, None, "B")
            if "p3" in PARTS:
                phase3(meanTB, t_xdTB, sb_w["wp1l"], sb_w["wp1r"],
                       sb_w["bp1"], sb_w["w2B"], 1, outs["ru"], "B")

            if "cc" not in PARTS:
                pass
            elif local_mode:
                nc.sync.dma_start(out=t_zcf[0:S, :], in_=st_zc[0:S, :])
            else:
                nc.gpsimd.collective_compute(
                    "AllGather", mybir.AluOpType.bypass,
                    replica_groups=[list(range(NCORES))],
                    ins=[st_zc[0:S, :]], outs=[t_zcf[0:cfg.N, :]])

            sb_g2A = gp.tile([128, GMAX], i16, tag="gidx", name="g2A")
            sb_g2B = gp.tile([128, GMAX], i16, tag="gidx", name="g2B")
            nc.sync.dma_start(out=sb_g2A[:, : ntA * 8], in_=t_g["g2A"][:])
            nc.sync.dma_start(out=sb_g2B[:, : ntB * 8], in_=t_g["g2B"][:])

            if "agg2" in PARTS:
                m2A = bigp.tile([128, SP], f16, tag="big", name="m2A")
                agg_pass(sb_g2A, TA, c2A, t_zcf[CENTER:, :], sb_r8iA,
                         m2A, CLS, 0, "A2")
                nc.sync.dma_start(out=outs["mu"][:], in_=m2A[0:CLS, :])
                m2B = bigp.tile([128, SP], f16, tag="big", name="m2B")
                agg_pass(sb_g2B, TB, c2B, t_zcf[CENTER:, :], sb_r8iB,
                         m2B, CLS, 1, "B2")
                nc.sync.dma_start(out=outs["mp"][:], in_=m2B[0:CLS, :])

    nc.finalize()
    return nc


def build(inputs, cfg=None, local_mode=False):
    cfg = cfg or CFG()
    in_maps, schedA, schedB, host_ctx = _prep_all(inputs, cfg)
    nc = _build_nc(cfg, schedA, schedB, local_mode=local_mode)
    return nc, in_maps, host_ctx


def assemble(results, host_ctx, cfg=None):
    """Un-permute per-core outputs and apply layer-2 biases (host side)."""
    cfg = cfg or CFG()
    S = cfg.S
    xu2 = np.zeros((cfg.N, CLS), np.float32)
    xp2 = np.zeros((cfg.N, CLS), np.float32)
    for c in range(NCORES):
        piA, piB = host_ctx["piA"][c], host_ctx["piB"][c]
        r = results[c]
        xu2[c * S + piA] = r["mu"].astype(np.float32).T[:S]
        xu2[c * S + piB] += r["ru"].astype(np.float32).T[:S]
        xp2[c * S + piB] = r["mp"].astype(np.float32).T[:S]
        xp2[c * S + piA] += r["rp"].astype(np.float32).T[:S]
    xu2 += host_ctx["b_u2"][None, :]
    xp2 += host_ctx["b_p2"][None, :]
    return xu2, xp2


def kernel(**inputs):
    from concourse.bass_utils import run_bass_kernel_spmd

    cfg = CFG()
    nc, in_maps, host_ctx = build(inputs, cfg)
    res = run_bass_kernel_spmd(nc, in_maps, list(range(NCORES)))
    return assemble(res.results, host_ctx, cfg)
